# revision 16
# baseline (speedup 1.0000x reference)
"""Fused masked-attention kernel for Trainium2, data-parallel over batch on 8 cores.

v18 design notes (all per core; one batch element per core):
- Steady state identical to the proven v15 shape: per tile (chunk pair)
  score-pair matmuls (row-split K=64 concurrent), ACT exp (the wall,
  ~1.13us/tile), DVE f16 2x-mode mask multiply, serial AV accumulation
  into one PSUM bank. All-f16 mask: measured power throttling (activity_1
  caps engine util at 50%) punishes extra engine work (Pool casts, 1x-mode
  DVE fp8 multiplies) more than the DMA bytes cost.
- Startup streams instead of serializing: only K0,K1 + V0,V1 + q0 load
  before the main loop (issued ahead of the mask DMA); K blocks 2..7 and
  V blocks 1..7 DMA+project inside qb0's slots just ahead of first use,
  so the exp stream starts ~15us in instead of ~50us.
- Output ships unnormalized O^T+Z rows; host does divide+transpose.
"""

import numpy as np

import concourse.bass as bass
import concourse.tile as tile
from concourse import bacc, mybir
from concourse import bass_utils

B, L, E, H = 8, 4096, 1024, 64
NCORES = 8
F32 = mybir.dt.float32
F16 = mybir.dt.float16

LB = 512           # q-block and projection block width
NQB = L // LB      # 8
NCH = L // 128     # 32 k-chunks
NEC = E // 128     # 8 e-chunks
NG = 16            # tiles (chunk pairs) per q-block


def build_nc():
    nc = bacc.Bacc(
        "TRN2",
        target_bir_lowering=False,
        debug=False,
        enable_asserts=False,
        num_devices=NCORES,
    )
    q2 = nc.dram_tensor("q2", [NQB, 128, NEC, LB], F16, kind="ExternalInput").ap()
    k2 = nc.dram_tensor("k2", [NQB, 128, NEC, LB], F16, kind="ExternalInput").ap()
    v2 = nc.dram_tensor("v2", [NQB, 128, NEC, LB], F16, kind="ExternalInput").ap()
    mu8 = nc.dram_tensor("mu8", [NQB, 128, NCH, LB], F16, kind="ExternalInput").ap()
    wqD = nc.dram_tensor("wqD", [E, 128], F16, kind="ExternalInput").ap()
    wkD = nc.dram_tensor("wkD", [E, 128], F16, kind="ExternalInput").ap()
    wvT = nc.dram_tensor("wvT", [E, H], F16, kind="ExternalInput").ap()
    ident = nc.dram_tensor("ident", [64, 64], F16, kind="ExternalInput").ap()
    out = nc.dram_tensor("out", [H + 1, L], F32, kind="ExternalOutput").ap()

    EXP = mybir.ActivationFunctionType.Exp

    with tile.TileContext(nc) as tc:
        with (
            tc.tile_pool(name="const", bufs=1) as constp,
            tc.tile_pool(name="persist", bufs=1) as persist,
            tc.tile_pool(name="kin", bufs=3) as kinp,
            tc.tile_pool(name="vin", bufs=2) as vinp,
            tc.tile_pool(name="qin", bufs=2) as qinp,
            tc.tile_pool(name="mpk", bufs=2) as mpool,
            tc.tile_pool(name="pt", bufs=10) as ptpool,
            tc.tile_pool(name="osb", bufs=2) as opool,
            tc.tile_pool(name="ps_st", bufs=2, space="PSUM") as ps_st,
            tc.tile_pool(name="ps_o", bufs=1, space="PSUM") as ps_o,
            tc.tile_pool(name="ps_pj", bufs=2, space="PSUM") as ps_pj,
        ):
            # ---- constants / weights ----
            # weights/masks/q go through the (otherwise idle) Pool engine's
            # DMA queue so they never head-of-line-block the K/V stream on SP
            wq_sb = constp.tile([128, NEC, 128], F16)
            wk_sb = constp.tile([128, NEC, 128], F16)
            wv_sb = constp.tile([128, NEC, H], F16)
            nc.gpsimd.dma_start(wq_sb[:], wqD.rearrange("(c p) h -> p c h", p=128))
            nc.gpsimd.dma_start(wk_sb[:], wkD.rearrange("(c p) h -> p c h", p=128))
            nc.gpsimd.dma_start(wv_sb[:], wvT.rearrange("(c p) h -> p c h", p=128))
            id_sb = constp.tile([64, 64], F16)
            nc.gpsimd.dma_start(id_sb[:], ident)

            # persistent projected tensors
            QT_sb = persist.tile([128, L], F16)   # rows 0:64 = Q^T, 64:128 copy
            KT_sb = persist.tile([128, L], F16)
            VT_sb = persist.tile([64, L], F16)    # V^T staging
            V_sb = persist.tile([128, NCH, 128], F16)  # [k, h] + ones col 64
            nc.vector.memset(V_sb[:, :, H : 128], 0.0)
            nc.vector.memset(V_sb[:, :, H : H + 1], 1.0)

            def proj_k_block(b, k_in):
                ls = b * LB
                p_pj = ps_pj.tile([128, LB], F32, tag="pj")
                for ec in range(NEC):
                    nc.tensor.matmul(
                        p_pj[:], wk_sb[:, ec, :], k_in[:, ec, :],
                        start=(ec == 0), stop=(ec == NEC - 1),
                    )
                nc.vector.tensor_copy(KT_sb[:, ls : ls + LB], p_pj[:])

            def proj_q_block(b, q_in):
                ls = b * LB
                p_pj = ps_pj.tile([128, LB], F32, tag="pj")
                for ec in range(NEC):
                    nc.tensor.matmul(
                        p_pj[:], wq_sb[:, ec, :], q_in[:, ec, :],
                        start=(ec == 0), stop=(ec == NEC - 1),
                    )
                nc.vector.tensor_copy(QT_sb[:, ls : ls + LB], p_pj[:])

            def proj_v_block(b, v_in):
                ls = b * LB
                p_pj = ps_pj.tile([128, LB], F32, tag="pj")
                for ec in range(NEC):
                    nc.tensor.matmul(
                        p_pj[0:H, :], wv_sb[:, ec, :], v_in[:, ec, :],
                        start=(ec == 0), stop=(ec == NEC - 1),
                    )
                nc.vector.tensor_copy(VT_sb[:, ls : ls + LB], p_pj[0:H, :])
                for sub in range(4):
                    c = b * 4 + sub
                    p_tr = ps_o.tile([128, H], F16, tag="pjt")
                    nc.tensor.transpose(
                        p_tr[:], VT_sb[:, c * 128 : (c + 1) * 128], id_sb[:]
                    )
                    nc.vector.tensor_copy(V_sb[:, c, 0:H], p_tr[:])

            def load_one(pool, tag, src, eng=None):
                t = pool.tile([128, NEC, LB], F16, tag=tag)
                (eng or nc.sync).dma_start(t[:], src)
                return t

            # ---- minimal startup: K0..K2 + V0,V1 on SP; q0 + mask0 on Pool ----
            q0 = load_one(qinp, "qin", q2[0], eng=nc.gpsimd)
            mpk_sb0 = mpool.tile([128, NCH, LB], F16, tag="mpk")
            nc.gpsimd.dma_start(mpk_sb0[:], mu8[0])
            k_in0 = load_one(kinp, "kin", k2[0])
            k_in1 = load_one(kinp, "kin", k2[1])
            k_in2 = load_one(kinp, "kin", k2[2])
            v_in0 = load_one(vinp, "vin", v2[0])
            v_in1 = load_one(vinp, "vin", v2[1])

            # ---- PE warmup on weights (HAM) ----
            p_w = ps_st.tile([128, 1024], F32, tag="p_st")
            for w in range(110):
                nc.tensor.matmul(
                    p_w[:, 0:128], wq_sb[:, 0, :], wq_sb[:, 0, 0:128],
                    start=True, stop=True,
                )

            proj_k_block(0, k_in0)
            proj_k_block(1, k_in1)
            proj_q_block(0, q0)
            proj_v_block(0, v_in0)

            # ---- main loop ----
            mtile = mpk_sb0
            q_next = None
            m_next = None
            k_pend = {2: k_in2}
            v_pend = {1: v_in1}
            for qb in range(NQB):
                qs = qb * LB
                p_o = ps_o.tile([128, LB], F32, tag="p_o")
                for g in range(NG):
                    cA, cB = 2 * g, 2 * g + 1
                    ps = ps_st.tile([128, 1024], F32, tag="p_st")
                    # two concurrent K=64 row-tiled score matmuls (N=512)
                    nc.tensor.matmul(
                        ps[:, 0:512],
                        KT_sb[0:64, cA * 128 : (cA + 1) * 128],
                        QT_sb[0:64, qs : qs + LB],
                        start=True, stop=True,
                    )
                    nc.tensor.matmul(
                        ps[:, 512:1024],
                        KT_sb[64:128, cB * 128 : (cB + 1) * 128],
                        QT_sb[64:128, qs : qs + LB],
                        start=True, stop=True,
                    )
                    # streaming projections during qb0
                    if qb == 0:
                        if g % 2 == 0:
                            b = g // 2 + 2
                            if b <= 7:
                                proj_k_block(b, k_pend.pop(b))
                            if b + 1 <= 7:
                                k_pend[b + 1] = load_one(kinp, "kin", k2[b + 1])
                        else:
                            b = (g + 1) // 2
                            if b <= 7:
                                proj_v_block(b, v_pend.pop(b))
                            if b + 1 <= 7:
                                v_pend[b + 1] = load_one(vinp, "vin", v2[b + 1])
                    # exp on ACT
                    pt = ptpool.tile([128, 1024], F16, tag="pt")
                    nc.scalar.activation(pt[:], ps[:], EXP, scale=0.125)
                    # mask-mult, all-f16 SBUF operands (DVE 2x mode)
                    nc.vector.tensor_mul(
                        pt[:],
                        pt[:],
                        mtile[:, cA : cA + 2, :].rearrange("p c q -> p (c q)"),
                    )
                    # prefetch hooks
                    if qb + 1 < NQB:
                        if g == 1:
                            m_next = mpool.tile([128, NCH, LB], F16, tag="mpk")
                            nc.gpsimd.dma_start(m_next[:], mu8[qb + 1])
                        if g == 4:
                            q_next = load_one(qinp, "qin", q2[qb + 1], eng=nc.gpsimd)
                        if g == 8:
                            proj_q_block(qb + 1, q_next)
                    # AV: accumulate both chunks
                    nc.tensor.matmul(
                        p_o[:], V_sb[:, cA, :], pt[:, 0:512],
                        start=(g == 0), stop=False,
                    )
                    nc.tensor.matmul(
                        p_o[:], V_sb[:, cB, :], pt[:, 512:1024],
                        start=False, stop=(g == NG - 1),
                    )
                # epilogue: ship unnormalized O^T + Z row
                o_sb = opool.tile([H + 1, LB], F32, tag="osb")
                nc.vector.tensor_copy(o_sb[:], p_o[0 : H + 1, :])
                nc.sync.dma_start(out[:, qs : qs + LB], o_sb[:])
                mtile = m_next
    nc.compile()
    return nc


_NC_CACHE = {}


def _shuffle_pcl(xT):
    """xT: [E, L] -> [NQB, 128, NEC, LB]."""
    a = xT.reshape(NEC, 128, NQB, LB)
    return np.ascontiguousarray(a.transpose(2, 1, 0, 3))


def _shuffle_mask(forb_b):
    """forb_b: [L, L] bool (True = forbidden) -> [NQB, 128, NCH, LB] u8."""
    A = forb_b.T.reshape(NCH, 128, NQB, LB)
    return np.ascontiguousarray(A.transpose(2, 1, 0, 3)).astype(np.uint8)


def kernel(query, key, value, mask, WQ, WK, WV):
    if "nc" not in _NC_CACHE:
        _NC_CACHE["nc"] = build_nc()
    nc = _NC_CACHE["nc"]

    wqT = np.asarray(WQ, dtype=np.float16).T  # [E, H]
    wkT = np.asarray(WK, dtype=np.float16).T
    wvT = np.ascontiguousarray(np.asarray(WV, dtype=np.float16).T)
    wqD = np.ascontiguousarray(np.concatenate([wqT, wqT], axis=1))
    wkD = np.ascontiguousarray(np.concatenate([wkT, wkT], axis=1))
    idn = np.eye(64, dtype=np.float16)
    forb = np.asarray(mask)  # [B, L, L], True where forbidden
    in_maps = []
    for b in range(B):
        in_maps.append(
            {
                "q2": _shuffle_pcl(np.asarray(query[b], dtype=np.float16).T),
                "k2": _shuffle_pcl(np.asarray(key[b], dtype=np.float16).T),
                "v2": _shuffle_pcl(np.asarray(value[b], dtype=np.float16).T),
                "mu8": (1 - _shuffle_mask(forb[b])).astype(np.float16),
                "wqD": wqD,
                "wkD": wkD,
                "wvT": wvT,
                "ident": idn,
            }
        )
    res = bass_utils.run_bass_kernel_spmd(nc, in_maps, core_ids=list(range(NCORES)))
    outs = []
    for b in range(B):
        ot = res.results[b]["out"].astype(np.float64)  # [65, L]
        o = (ot[0:H] / ot[H : H + 1]).T  # [L, H]
        outs.append(o.astype(np.float32))
    return np.stack(outs, axis=0)


if __name__ == "__main__":
    rng = np.random.default_rng(0)
    q = rng.standard_normal((B, L, E), dtype=np.float32)
    k = rng.standard_normal((B, L, E), dtype=np.float32)
    v = rng.standard_normal((B, L, E), dtype=np.float32)
    m = rng.integers(0, 2, size=(B, L, L)).astype(bool)
    s = 1.0 / np.sqrt(E)
    wq = rng.uniform(-s, s, size=(H, E)).astype(np.float32)
    wk = rng.uniform(-s, s, size=(H, E)).astype(np.float32)
    wv = rng.uniform(-s, s, size=(H, E)).astype(np.float32)
    o = kernel(query=q, key=k, value=v, mask=m, WQ=wq, WK=wk, WV=wv)
    print(o.shape, o.dtype)


# revision 18
# speedup vs baseline: 1.0912x; 1.0912x over previous
"""Fused masked-attention kernel for Trainium2, data-parallel over batch on 8 cores.

v20 design notes (all per core; one batch element per core):
- Steady state: per tile (chunk pair) score-pair matmuls (row-split K=64
  concurrent), ACT exp (the wall, ~1.13us/tile), DVE f16 2x-mode mask
  multiply, serial AV accumulation into one PSUM bank. Measured power
  throttling (activity_1 caps util at 50%) punishes extra engine work, so
  steady state stays minimal-energy: no Pool casts, no PE mask matmuls.
- Ramp optimizations (the wall is the first ~90us, which is DMA-faucet
  bound at ~430GB/s):
    * PE warmup (HAM) runs on a memset tile at t~0 instead of waiting for
      the weights DMA, so projections start the moment K0 lands.
    * qb0/qb1 masks ship as fp8e4 {0,1} (2.1MB instead of 8.4MB inside the
      critical window) and are applied by direct DVE fp8 multiplies (1x) --
      DVE is otherwise starved during the ramp. qb2..7 masks stay f16.
    * K gets strict DMA priority (K0..K2 head the SP queue; fp8 mask halves
      interleave so the first tiles' masks land just in time).
- Output ships unnormalized O^T+Z rows; host does divide+transpose.
"""

import numpy as np
import ml_dtypes

import concourse.bass as bass
import concourse.tile as tile
from concourse import bacc, mybir
from concourse import bass_utils

B, L, E, H = 8, 4096, 1024, 64
NCORES = 8
F32 = mybir.dt.float32
F16 = mybir.dt.float16
F8 = mybir.dt.float8e4

LB = 512           # q-block and projection block width
NQB = L // LB      # 8
NCH = L // 128     # 32 k-chunks
NEC = E // 128     # 8 e-chunks
NG = 16            # tiles (chunk pairs) per q-block
NQB8 = 2           # leading q-blocks whose mask ships fp8 (ramp relief)


def build_nc():
    nc = bacc.Bacc(
        "TRN2",
        target_bir_lowering=False,
        debug=False,
        enable_asserts=False,
        num_devices=NCORES,
    )
    q2 = nc.dram_tensor("q2", [NQB, 128, NEC, LB], F16, kind="ExternalInput").ap()
    k2 = nc.dram_tensor("k2", [NQB, 128, NEC, LB], F16, kind="ExternalInput").ap()
    v2 = nc.dram_tensor("v2", [NQB, 128, NEC, LB], F16, kind="ExternalInput").ap()
    m8 = nc.dram_tensor("m8", [NQB8, 128, NCH, LB], F8, kind="ExternalInput").ap()
    mu8 = nc.dram_tensor(
        "mu8", [NQB - NQB8, 128, NCH, LB], F16, kind="ExternalInput"
    ).ap()
    wqD = nc.dram_tensor("wqD", [E, 128], F16, kind="ExternalInput").ap()
    wkD = nc.dram_tensor("wkD", [E, 128], F16, kind="ExternalInput").ap()
    wvT = nc.dram_tensor("wvT", [E, H], F16, kind="ExternalInput").ap()
    ident = nc.dram_tensor("ident", [64, 64], F16, kind="ExternalInput").ap()
    out = nc.dram_tensor("out", [H + 1, L], F32, kind="ExternalOutput").ap()

    EXP = mybir.ActivationFunctionType.Exp

    with tile.TileContext(nc) as tc:
        with (
            tc.tile_pool(name="const", bufs=1) as constp,
            tc.tile_pool(name="persist", bufs=1) as persist,
            tc.tile_pool(name="kin", bufs=3) as kinp,
            tc.tile_pool(name="vin", bufs=2) as vinp,
            tc.tile_pool(name="qin", bufs=2) as qinp,
            tc.tile_pool(name="m8p", bufs=2) as m8pool,
            tc.tile_pool(name="mpk", bufs=2) as mpool,
            tc.tile_pool(name="pt", bufs=7) as ptpool,
            tc.tile_pool(name="osb", bufs=2) as opool,
            tc.tile_pool(name="ps_st", bufs=2, space="PSUM") as ps_st,
            tc.tile_pool(name="ps_o", bufs=1, space="PSUM") as ps_o,
            tc.tile_pool(name="ps_pj", bufs=2, space="PSUM") as ps_pj,
        ):
            # ---- warmup constant, ready with no DMA dependency ----
            wu = constp.tile([128, 128], F16)
            nc.vector.memset(wu[:], 0.01)

            # ---- PE warmup (HAM) on the memset tile, t ~ 0 ----
            p_w = ps_st.tile([128, 1024], F32, tag="p_st")
            for w in range(80):
                nc.tensor.matmul(
                    p_w[:, 0:128], wu[:], wu[:], start=True, stop=True,
                )

            # ---- constants / weights ----
            wq_sb = constp.tile([128, NEC, 128], F16)
            wk_sb = constp.tile([128, NEC, 128], F16)
            wv_sb = constp.tile([128, NEC, H], F16)
            nc.sync.dma_start(wq_sb[:], wqD.rearrange("(c p) h -> p c h", p=128))
            nc.sync.dma_start(wk_sb[:], wkD.rearrange("(c p) h -> p c h", p=128))
            nc.sync.dma_start(wv_sb[:], wvT.rearrange("(c p) h -> p c h", p=128))
            id_sb = constp.tile([64, 64], F16)
            nc.sync.dma_start(id_sb[:], ident)

            # persistent projected tensors
            QT_sb = persist.tile([128, L], F16)   # rows 0:64 = Q^T, 64:128 copy
            KT_sb = persist.tile([128, L], F16)
            VT_sb = persist.tile([64, L], F16)    # V^T staging
            V_sb = persist.tile([128, NCH, 128], F16)  # [k, h] + ones col 64
            nc.vector.memset(V_sb[:, :, H : 128], 0.0)
            nc.vector.memset(V_sb[:, :, H : H + 1], 1.0)

            def proj_k_block(b, k_in):
                ls = b * LB
                p_pj = ps_pj.tile([128, LB], F32, tag="pj")
                for ec in range(NEC):
                    nc.tensor.matmul(
                        p_pj[:], wk_sb[:, ec, :], k_in[:, ec, :],
                        start=(ec == 0), stop=(ec == NEC - 1),
                    )
                nc.vector.tensor_copy(KT_sb[:, ls : ls + LB], p_pj[:])

            def proj_q_block(b, q_in):
                ls = b * LB
                p_pj = ps_pj.tile([128, LB], F32, tag="pj")
                for ec in range(NEC):
                    nc.tensor.matmul(
                        p_pj[:], wq_sb[:, ec, :], q_in[:, ec, :],
                        start=(ec == 0), stop=(ec == NEC - 1),
                    )
                nc.vector.tensor_copy(QT_sb[:, ls : ls + LB], p_pj[:])

            def proj_v_block(b, v_in):
                ls = b * LB
                p_pj = ps_pj.tile([128, LB], F32, tag="pj")
                for ec in range(NEC):
                    nc.tensor.matmul(
                        p_pj[0:H, :], wv_sb[:, ec, :], v_in[:, ec, :],
                        start=(ec == 0), stop=(ec == NEC - 1),
                    )
                nc.vector.tensor_copy(VT_sb[:, ls : ls + LB], p_pj[0:H, :])
                for sub in range(4):
                    c = b * 4 + sub
                    p_tr = ps_o.tile([128, H], F16, tag="pjt")
                    nc.tensor.transpose(
                        p_tr[:], VT_sb[:, c * 128 : (c + 1) * 128], id_sb[:]
                    )
                    nc.vector.tensor_copy(V_sb[:, c, 0:H], p_tr[:])

            def load_one(pool, tag, src):
                t = pool.tile([128, NEC, LB], F16, tag=tag)
                nc.sync.dma_start(t[:], src)
                return t

            # ---- startup: K first, fp8 mask halves interleaved ----
            k_in0 = load_one(kinp, "kin", k2[0])
            k_in1 = load_one(kinp, "kin", k2[1])
            q0 = load_one(qinp, "qin", q2[0])
            m8_sb0 = m8pool.tile([128, NCH, LB], F8, tag="m8")
            nc.sync.dma_start(m8_sb0[:, 0:16, :], m8[0, :, 0:16, :])
            k_in2 = load_one(kinp, "kin", k2[2])
            v_in0 = load_one(vinp, "vin", v2[0])
            nc.sync.dma_start(m8_sb0[:, 16:NCH, :], m8[0, :, 16:NCH, :])
            v_in1 = load_one(vinp, "vin", v2[1])

            proj_k_block(0, k_in0)
            proj_k_block(1, k_in1)
            proj_q_block(0, q0)
            proj_v_block(0, v_in0)

            # ---- main loop ----
            mtile = m8_sb0
            q_next = None
            m_next = None
            k_pend = {2: k_in2}
            v_pend = {1: v_in1}
            for qb in range(NQB):
                qs = qb * LB
                fp8_qb = qb < NQB8
                p_o = ps_o.tile([128, LB], F32, tag="p_o")
                for g in range(NG):
                    cA, cB = 2 * g, 2 * g + 1
                    ps = ps_st.tile([128, 1024], F32, tag="p_st")
                    # two concurrent K=64 row-tiled score matmuls (N=512)
                    nc.tensor.matmul(
                        ps[:, 0:512],
                        KT_sb[0:64, cA * 128 : (cA + 1) * 128],
                        QT_sb[0:64, qs : qs + LB],
                        start=True, stop=True,
                    )
                    nc.tensor.matmul(
                        ps[:, 512:1024],
                        KT_sb[64:128, cB * 128 : (cB + 1) * 128],
                        QT_sb[64:128, qs : qs + LB],
                        start=True, stop=True,
                    )
                    # streaming projections during qb0
                    if qb == 0:
                        if g % 2 == 0:
                            b = g // 2 + 2
                            if b <= 7:
                                proj_k_block(b, k_pend.pop(b))
                            if b + 1 <= 7:
                                k_pend[b + 1] = load_one(kinp, "kin", k2[b + 1])
                        else:
                            b = (g + 1) // 2
                            if b <= 7:
                                proj_v_block(b, v_pend.pop(b))
                            if b + 1 <= 7:
                                v_pend[b + 1] = load_one(vinp, "vin", v2[b + 1])
                    # exp on ACT
                    pt = ptpool.tile([128, 1024], F16, tag="pt")
                    nc.scalar.activation(pt[:], ps[:], EXP, scale=0.125)
                    # mask-mult on DVE (fp8 1x during ramp, f16 2x steady)
                    nc.vector.tensor_mul(
                        pt[:],
                        pt[:],
                        mtile[:, cA : cA + 2, :].rearrange("p c q -> p (c q)"),
                    )
                    # prefetch hooks
                    if qb + 1 < NQB:
                        if g == 1:
                            if qb + 1 < NQB8:
                                m_next = m8pool.tile([128, NCH, LB], F8, tag="m8")
                                nc.sync.dma_start(m_next[:], m8[qb + 1])
                            else:
                                m_next = mpool.tile([128, NCH, LB], F16, tag="mpk")
                                nc.sync.dma_start(m_next[:], mu8[qb + 1 - NQB8])
                        if g == 4:
                            q_next = load_one(qinp, "qin", q2[qb + 1])
                        if g == 8:
                            proj_q_block(qb + 1, q_next)
                    # AV: accumulate both chunks
                    nc.tensor.matmul(
                        p_o[:], V_sb[:, cA, :], pt[:, 0:512],
                        start=(g == 0), stop=False,
                    )
                    nc.tensor.matmul(
                        p_o[:], V_sb[:, cB, :], pt[:, 512:1024],
                        start=False, stop=(g == NG - 1),
                    )
                # epilogue: ship unnormalized O^T + Z row
                o_sb = opool.tile([H + 1, LB], F32, tag="osb")
                nc.vector.tensor_copy(o_sb[:], p_o[0 : H + 1, :])
                nc.sync.dma_start(out[:, qs : qs + LB], o_sb[:])
                mtile = m_next
    nc.compile()
    return nc


_NC_CACHE = {}


def _shuffle_pcl(xT):
    """xT: [E, L] -> [NQB, 128, NEC, LB]."""
    a = xT.reshape(NEC, 128, NQB, LB)
    return np.ascontiguousarray(a.transpose(2, 1, 0, 3))


def _shuffle_mask(forb_b):
    """forb_b: [L, L] bool (True = forbidden) -> [NQB, 128, NCH, LB] u8
    allowed mask: [qb, p, c, q'] = 1 - forb[qb*512+q', c*128+p]."""
    A = forb_b.T.reshape(NCH, 128, NQB, LB)
    return (1 - np.ascontiguousarray(A.transpose(2, 1, 0, 3))).astype(np.uint8)


def kernel(query, key, value, mask, WQ, WK, WV):
    if "nc" not in _NC_CACHE:
        _NC_CACHE["nc"] = build_nc()
    nc = _NC_CACHE["nc"]

    wqT = np.asarray(WQ, dtype=np.float16).T  # [E, H]
    wkT = np.asarray(WK, dtype=np.float16).T
    wvT = np.ascontiguousarray(np.asarray(WV, dtype=np.float16).T)
    wqD = np.ascontiguousarray(np.concatenate([wqT, wqT], axis=1))
    wkD = np.ascontiguousarray(np.concatenate([wkT, wkT], axis=1))
    idn = np.eye(64, dtype=np.float16)
    forb = np.asarray(mask)  # [B, L, L], True where forbidden
    in_maps = []
    for b in range(B):
        allow = _shuffle_mask(forb[b])  # [NQB, 128, NCH, LB] u8 {0,1}
        m8b = np.where(allow[:NQB8] > 0, np.uint8(0x38), np.uint8(0)).view(
            ml_dtypes.float8_e4m3
        )
        in_maps.append(
            {
                "q2": _shuffle_pcl(np.asarray(query[b], dtype=np.float16).T),
                "k2": _shuffle_pcl(np.asarray(key[b], dtype=np.float16).T),
                "v2": _shuffle_pcl(np.asarray(value[b], dtype=np.float16).T),
                "m8": np.ascontiguousarray(m8b),
                "mu8": allow[NQB8:].astype(np.float16),
                "wqD": wqD,
                "wkD": wkD,
                "wvT": wvT,
                "ident": idn,
            }
        )
    res = bass_utils.run_bass_kernel_spmd(nc, in_maps, core_ids=list(range(NCORES)))
    outs = []
    for b in range(B):
        ot = res.results[b]["out"].astype(np.float64)  # [65, L]
        o = (ot[0:H] / ot[H : H + 1]).T  # [L, H]
        outs.append(o.astype(np.float32))
    return np.stack(outs, axis=0)


if __name__ == "__main__":
    rng = np.random.default_rng(0)
    q = rng.standard_normal((B, L, E), dtype=np.float32)
    k = rng.standard_normal((B, L, E), dtype=np.float32)
    v = rng.standard_normal((B, L, E), dtype=np.float32)
    m = rng.integers(0, 2, size=(B, L, L)).astype(bool)
    s = 1.0 / np.sqrt(E)
    wq = rng.uniform(-s, s, size=(H, E)).astype(np.float32)
    wk = rng.uniform(-s, s, size=(H, E)).astype(np.float32)
    wv = rng.uniform(-s, s, size=(H, E)).astype(np.float32)
    o = kernel(query=q, key=k, value=v, mask=m, WQ=wq, WK=wk, WV=wv)
    print(o.shape, o.dtype)
